# revision 10
# baseline (speedup 1.0000x reference)
"""Trainium2 Bass kernel for OfficeLSTM:
  h = LSTM(x)[last];  out = softmax(relu(h @ W1 + b1) @ W2 + b2)

Shapes: x [512, 256, 256] f32, Wx [256, 2048], Wh [512, 2048], b [2048],
W1 [512, 128], b1 [128], W2 [128, 10], b2 [10].  Output [512, 10] f32.

Strategy: data-parallel over 8 NeuronCores (64 batch rows each); weights
replicated (prepacked on host to bf16, gate-permuted to [i,f,o,g]).
Per-core layout keeps gates on SBUF partitions:
  z.T [2048, 64] = Wh.T @ h.T (+ xz.T), computed as 16 M-tiles x 4 K-tiles
  of [128,128]x[128,64] bf16 matmuls with Wh stationary (FWL active).
x-projections are precomputed per 8-step window with N=512 matmuls from
PE-transposed x tiles; bias is folded in via the ScalarE copy that drains
the window PSUM.  Cell state c stays f32; h is bf16.
"""

import numpy as np
import ml_dtypes
from contextlib import ExitStack

import concourse.bacc as bacc
import concourse.tile as tile
import concourse.mybir as mybir
from concourse.bass_utils import run_bass_kernel_spmd

F32 = mybir.dt.float32
BF16 = mybir.dt.bfloat16
AF = mybir.ActivationFunctionType
ALU = mybir.AluOpType

NCORES = 8
B_TOT, T, D = 512, 256, 256
U = 512
G = 4 * U            # 2048
B = B_TOT // NCORES  # 64 per core
W = 8                # window length (timesteps of xz precomputed per batch)
NW = T // W

# gate order inside the 2048 dim after host permutation: i, f, o, g
_GATE_PERM = [0, 1, 3, 2]  # new block -> original gate block


def _build(t_steps=T):
    nc = bacc.Bacc("TRN2", target_bir_lowering=False, debug=False)

    x_d = nc.declare_dram_parameter("x", [B, T, D], F32, isOutput=False)
    whb_d = nc.declare_dram_parameter("whb", [128, 4 * G], BF16, isOutput=False)
    wxb_d = nc.declare_dram_parameter("wxb", [128, 2 * G], BF16, isOutput=False)
    w1b_d = nc.declare_dram_parameter("w1b", [128, 512], BF16, isOutput=False)
    w2b_d = nc.declare_dram_parameter("w2b", [128, 10], BF16, isOutput=False)
    bvec_d = nc.declare_dram_parameter("bvec", [128, 16], F32, isOutput=False)
    b1v_d = nc.declare_dram_parameter("b1v", [128, 1], F32, isOutput=False)
    b2v_d = nc.declare_dram_parameter("b2v", [10, 1], F32, isOutput=False)
    id128_d = nc.declare_dram_parameter("id128", [128, 128], F32, isOutput=False)
    idbf_d = nc.declare_dram_parameter("idbf", [128, 128], BF16, isOutput=False)
    id10_d = nc.declare_dram_parameter("id10", [10, 10], F32, isOutput=False)
    out_d = nc.declare_dram_parameter("out", [B, 10], F32, isOutput=True)

    with tile.TileContext(nc) as tc:
        with ExitStack() as ctx:
            const = ctx.enter_context(tc.tile_pool(name="const", bufs=1))
            state = ctx.enter_context(tc.tile_pool(name="state", bufs=1))
            hpool = ctx.enter_context(tc.tile_pool(name="hpool", bufs=2))
            xwp = ctx.enter_context(tc.tile_pool(name="xwp", bufs=8))
            xtp = ctx.enter_context(tc.tile_pool(name="xtp", bufs=4))
            xzp = ctx.enter_context(tc.tile_pool(name="xzp", bufs=2))
            zp = ctx.enter_context(tc.tile_pool(name="zp", bufs=4))
            tmpp = ctx.enter_context(tc.tile_pool(name="tmpp", bufs=4))
            headp = ctx.enter_context(tc.tile_pool(name="headp", bufs=1))
            pstep = ctx.enter_context(tc.tile_pool(name="pstep", bufs=4, space="PSUM"))
            pwin = ctx.enter_context(tc.tile_pool(name="pwin", bufs=2, space="PSUM"))
            ptr = ctx.enter_context(tc.tile_pool(name="ptr", bufs=2, space="PSUM"))

            # ---- constants ----
            whb = const.tile([128, 4 * G], BF16, name="whb_s")
            nc.sync.dma_start(whb[:], whb_d[:])
            wxb = const.tile([128, 2 * G], BF16, name="wxb_s")
            nc.sync.dma_start(wxb[:], wxb_d[:])
            w1b = const.tile([128, 512], BF16, name="w1b_s")
            nc.sync.dma_start(w1b[:], w1b_d[:])
            w2b = const.tile([128, 10], BF16, name="w2b_s")
            nc.sync.dma_start(w2b[:], w2b_d[:])
            bvec = const.tile([128, 16], F32, name="bvec_s")
            nc.sync.dma_start(bvec[:], bvec_d[:])
            b1v = const.tile([128, 1], F32, name="b1v_s")
            nc.sync.dma_start(b1v[:], b1v_d[:])
            b2v = const.tile([10, 1], F32, name="b2v_s")
            nc.sync.dma_start(b2v[:], b2v_d[:])
            id128 = const.tile([128, 128], F32, name="id128_s")
            nc.sync.dma_start(id128[:], id128_d[:])
            id10 = const.tile([10, 10], F32, name="id10_s")
            nc.sync.dma_start(id10[:], id10_d[:])

            # ---- state ----
            c = state.tile([128, 256], F32, name="c_s")
            nc.vector.memset(c[:], 0.0)
            h_cur = hpool.tile([128, 256], BF16, name="h_init", tag="h")
            nc.vector.memset(h_cur[:], 0.0)



            def emit_window(w):
                """Compute xzb[w]: bf16 [128, 16*512]; col m*512 + s*64 + b holds
                (Wx.T x_t + b) for gate-row m*128+p, window step s, batch b."""
                xts = [
                    xtp.tile([128, 512], BF16, name=f"xt{dk}_{w}", tag=f"xt{dk}")
                    for dk in range(2)
                ]
                for q in range(4):
                    xw = xwp.tile([128, 256], F32, name=f"xw{w}_{q}", tag="xw")
                    for two in range(2):
                        tt = w * W + 2 * q + two
                        nc.sync.dma_start(
                            xw[two * 64 : (two + 1) * 64, :], x_d[:, tt, :]
                        )
                    for dk in range(2):
                        ptile = ptr.tile(
                            [128, 128], F32, name=f"ptr{w}_{q}_{dk}", tag="ptr"
                        )
                        nc.tensor.transpose(
                            ptile[:], xw[:, dk * 128 : (dk + 1) * 128], id128[:]
                        )
                        nc.vector.tensor_copy(
                            xts[dk][:, q * 128 : (q + 1) * 128], ptile[:]
                        )
                xzb = xzp.tile([128, 16 * 512], BF16, name=f"xzb{w}", tag="xzb")
                for m in range(16):
                    pw = pwin.tile([128, 512], F32, name=f"pw{w}_{m}", tag="pw")
                    for k in range(2):
                        nc.tensor.matmul(
                            pw[:],
                            wxb[:, k * G + m * 128 : k * G + (m + 1) * 128],
                            xts[k][:],
                            start=(k == 0),
                            stop=(k == 1),
                        )
                    nc.scalar.activation(
                        xzb[:, m * 512 : (m + 1) * 512],
                        pw[:],
                        AF.Identity,
                        bias=bvec[:, m : m + 1],
                    )
                return xzb

            def emit_step(t, xzb, h_prev, h_new):
                s = t % W
                for grp in range(2):
                    j0 = 2 * grp
                    ps = pstep.tile([128, 512], F32, name=f"ps{t}_{grp}", tag="ps")
                    # Single accumulation group for the whole bank: the first
                    # matmul's start=True clears has_written bank-wide; every
                    # other matmul first-touches its region with start=False
                    # (overwrite+set) then accumulates.  k-outer order lets
                    # k=0,1 matmuls issue as soon as h chunks 0,1 are ready.
                    for k in range(4):
                        for gate in range(4):
                            for jl in range(2):
                                m = gate * 4 + j0 + jl
                                col = gate * 128 + jl * 64
                                nc.tensor.matmul(
                                    ps[:, col : col + 64],
                                    whb[:, k * G + m * 128 : k * G + (m + 1) * 128],
                                    h_prev[:, k * 64 : (k + 1) * 64],
                                    start=(k == 0 and gate == 0 and jl == 0),
                                    stop=(k == 3 and gate == 3 and jl == 1),
                                    skip_group_check=True,
                                )
                    # z = ps + xz, done in-place in PSUM, split ifo/g so the
                    # sigmoid can start while the g-block add still runs.
                    xvw = xzb.rearrange("p (g j sb) -> p g j sb", g=4, j=4)
                    pv = ps.rearrange("p (g j b) -> p g j b", g=4, j=2)
                    nc.vector.tensor_tensor(
                        pv[:, 0:3],
                        pv[:, 0:3],
                        xvw[:, 0:3, j0 : j0 + 2, s * 64 : (s + 1) * 64],
                        ALU.add,
                    )
                    nc.vector.tensor_tensor(
                        pv[:, 3:4],
                        pv[:, 3:4],
                        xvw[:, 3:4, j0 : j0 + 2, s * 64 : (s + 1) * 64],
                        ALU.add,
                    )
                    zg = zp.tile([128, 512], F32, name=f"z{t}_{grp}", tag="z")
                    nc.scalar.activation(zg[:, 0:384], ps[:, 0:384], AF.Sigmoid)
                    nc.scalar.activation(zg[:, 384:512], ps[:, 384:512], AF.Tanh)
                    cg = c[:, grp * 128 : (grp + 1) * 128]
                    tmp = tmpp.tile([128, 128], F32, name=f"tmp{t}_{grp}", tag="tmp")
                    nc.vector.tensor_mul(cg, cg, zg[:, 128:256])
                    nc.vector.tensor_mul(tmp[:], zg[:, 0:128], zg[:, 384:512])
                    nc.vector.tensor_add(cg, cg, tmp[:])
                    th = tmpp.tile([128, 128], F32, name=f"th{t}_{grp}", tag="th")
                    nc.scalar.activation(th[:], cg, AF.Tanh)
                    nc.vector.tensor_mul(
                        h_new[:, grp * 128 : (grp + 1) * 128], zg[:, 256:384], th[:]
                    )

            n_windows = (t_steps + W - 1) // W
            xzbs = [None] * n_windows
            xzbs[0] = emit_window(0)
            for t in range(t_steps):
                w, s = divmod(t, W)
                if s == 0 and w + 1 < n_windows:
                    xzbs[w + 1] = emit_window(w + 1)
                h_new = hpool.tile([128, 256], BF16, name=f"h{t}", tag="h")
                emit_step(t, xzbs[w], h_cur, h_new)
                h_cur = h_new

            # ---- MLP head + softmax ----
            ps1 = ptr.tile([128, 64], F32, name="ps1", tag="ptr")
            for k in range(4):
                nc.tensor.matmul(
                    ps1[:],
                    w1b[:, k * 128 : (k + 1) * 128],
                    h_cur[:, k * 64 : (k + 1) * 64],
                    start=(k == 0),
                    stop=(k == 3),
                )
            h1 = headp.tile([128, 64], BF16, name="h1")
            nc.scalar.activation(h1[:], ps1[:], AF.Relu, bias=b1v[:, 0:1])
            ps2 = ptr.tile([10, 64], F32, name="ps2", tag="ptr")
            nc.tensor.matmul(ps2[:], w2b[:, 0:10], h1[:], start=True, stop=True)
            s2 = headp.tile([10, 64], F32, name="s2")
            nc.scalar.activation(s2[:], ps2[:], AF.Identity, bias=b2v[:, 0:1])
            ps3 = ptr.tile([64, 10], F32, name="ps3", tag="ptr")
            nc.tensor.transpose(ps3[:], s2[:], id10[:])
            z3 = headp.tile([64, 10], F32, name="z3")
            nc.vector.tensor_copy(z3[:], ps3[:])
            mx = headp.tile([64, 1], F32, name="mx")
            nc.vector.tensor_reduce(mx[:], z3[:], axis=mybir.AxisListType.X, op=ALU.max)
            nc.vector.tensor_scalar(z3[:], z3[:], mx[:, 0:1], None, ALU.subtract)
            e3 = headp.tile([64, 10], F32, name="e3")
            nc.scalar.activation(e3[:], z3[:], AF.Exp)
            sm = headp.tile([64, 1], F32, name="sm")
            nc.vector.tensor_reduce(sm[:], e3[:], axis=mybir.AxisListType.X, op=ALU.add)
            rc = headp.tile([64, 1], F32, name="rc")
            nc.vector.reciprocal(rc[:], sm[:])
            o3 = headp.tile([64, 10], F32, name="o3")
            nc.vector.tensor_scalar(o3[:], e3[:], rc[:, 0:1], None, ALU.mult)
            nc.sync.dma_start(out_d[:], o3[:])

    nc.compile()
    return nc


def _prepack(Wx, Wh, b, W1, b1, W2, b2):
    perm = np.concatenate([np.arange(512) + g * 512 for g in _GATE_PERM])
    bf = ml_dtypes.bfloat16
    Wh_p = np.ascontiguousarray(Wh[:, perm])
    Wx_p = np.ascontiguousarray(Wx[:, perm])
    b_p = np.ascontiguousarray(b[perm])
    whb = Wh_p.reshape(4, 128, G).transpose(1, 0, 2).reshape(128, 4 * G).astype(bf)
    wxb = Wx_p.reshape(2, 128, G).transpose(1, 0, 2).reshape(128, 2 * G).astype(bf)
    w1b = W1.reshape(4, 128, 128).transpose(1, 0, 2).reshape(128, 512).astype(bf)
    w2b = W2.astype(bf)
    bvec = np.ascontiguousarray(b_p.reshape(16, 128).T.astype(np.float32))
    b1v = b1.astype(np.float32).reshape(128, 1)
    b2v = b2.astype(np.float32).reshape(10, 1)
    id128 = np.eye(128, dtype=np.float32)
    id10 = np.eye(10, dtype=np.float32)
    return dict(
        whb=np.ascontiguousarray(whb),
        wxb=np.ascontiguousarray(wxb),
        w1b=np.ascontiguousarray(w1b),
        w2b=np.ascontiguousarray(w2b),
        bvec=bvec,
        b1v=b1v,
        b2v=b2v,
        id128=id128,
        id10=id10,
    )


_NC_CACHE = {}


def get_nc(t_steps=T):
    if t_steps not in _NC_CACHE:
        _NC_CACHE[t_steps] = _build(t_steps)
    return _NC_CACHE[t_steps]


def kernel(x, Wx, Wh, b, W1, b1, W2, b2, _trace=False):
    x = np.asarray(x, dtype=np.float32)
    consts = _prepack(
        np.asarray(Wx, np.float32),
        np.asarray(Wh, np.float32),
        np.asarray(b, np.float32),
        np.asarray(W1, np.float32),
        np.asarray(b1, np.float32),
        np.asarray(W2, np.float32),
        np.asarray(b2, np.float32),
    )
    nc = get_nc()
    in_maps = [
        {"x": np.ascontiguousarray(x[i * B : (i + 1) * B]), **consts}
        for i in range(NCORES)
    ]
    res = run_bass_kernel_spmd(nc, in_maps, core_ids=list(range(NCORES)), trace=_trace)
    out = np.concatenate([res.results[i]["out"] for i in range(NCORES)], axis=0)
    if _trace:
        return out, res
    return out


# revision 15
# speedup vs baseline: 1.4250x; 1.4250x over previous
"""Trainium2 Bass kernel for OfficeLSTM:
  h = LSTM(x)[last];  out = softmax(relu(h @ W1 + b1) @ W2 + b2)

Shapes: x [512, 256, 256] f32, Wx [256, 2048], Wh [512, 2048], b [2048],
W1 [512, 128], b1 [128], W2 [128, 10], b2 [10].  Output [512, 10] f32.

Strategy: data-parallel over 8 NeuronCores (64 batch rows each); weights
replicated (prepacked on host to bf16, gate-permuted to [i,f,o,g]).
Per-core layout keeps gates on SBUF partitions:
  z.T [2048, 64] = Wh.T @ h.T (+ xz.T), computed as 16 M-tiles x 4 K-tiles
  of [128,128]x[128,64] bf16 matmuls with Wh stationary (FWL active).
x-projections are precomputed per 8-step window with N=512 matmuls from
PE-transposed x tiles; bias is folded in via the ScalarE copy that drains
the window PSUM.  Cell state c stays f32; h is bf16.
"""

import numpy as np
import ml_dtypes
from contextlib import ExitStack

import concourse.bacc as bacc
import concourse.tile as tile
import concourse.mybir as mybir
from concourse.bass_utils import run_bass_kernel_spmd

F32 = mybir.dt.float32
BF16 = mybir.dt.bfloat16
AF = mybir.ActivationFunctionType
ALU = mybir.AluOpType

NCORES = 8
B_TOT, T, D = 512, 256, 256
U = 512
G = 4 * U            # 2048
B = B_TOT // NCORES  # 64 per core
W = 8                # window length (timesteps of xz precomputed per batch)
NW = T // W

# gate order inside the 2048 dim after host permutation: i, f, o, g
_GATE_PERM = [0, 1, 3, 2]  # new block -> original gate block


def _build(t_steps=T):
    nc = bacc.Bacc("TRN2", target_bir_lowering=False, debug=False)

    x_d = nc.declare_dram_parameter("x", [B, T, D], F32, isOutput=False)
    whb_d = nc.declare_dram_parameter("whb", [128, 4 * G], BF16, isOutput=False)
    wxb_d = nc.declare_dram_parameter("wxb", [128, 2 * G], BF16, isOutput=False)
    w1b_d = nc.declare_dram_parameter("w1b", [128, 512], BF16, isOutput=False)
    w2b_d = nc.declare_dram_parameter("w2b", [128, 10], BF16, isOutput=False)
    bvec_d = nc.declare_dram_parameter("bvec", [128, 16], F32, isOutput=False)
    b1v_d = nc.declare_dram_parameter("b1v", [128, 1], F32, isOutput=False)
    b2v_d = nc.declare_dram_parameter("b2v", [10, 1], F32, isOutput=False)
    id128_d = nc.declare_dram_parameter("id128", [128, 128], F32, isOutput=False)
    idbf_d = nc.declare_dram_parameter("idbf", [128, 128], BF16, isOutput=False)
    id10_d = nc.declare_dram_parameter("id10", [10, 10], F32, isOutput=False)
    out_d = nc.declare_dram_parameter("out", [B, 10], F32, isOutput=True)

    with tile.TileContext(nc) as tc:
        with ExitStack() as ctx:
            const = ctx.enter_context(tc.tile_pool(name="const", bufs=1))
            state = ctx.enter_context(tc.tile_pool(name="state", bufs=1))
            hpool = ctx.enter_context(tc.tile_pool(name="hpool", bufs=2))
            xwp = ctx.enter_context(tc.tile_pool(name="xwp", bufs=8))
            xtp = ctx.enter_context(tc.tile_pool(name="xtp", bufs=4))
            xzp = ctx.enter_context(tc.tile_pool(name="xzp", bufs=2))
            zp = ctx.enter_context(tc.tile_pool(name="zp", bufs=4))
            tmpp = ctx.enter_context(tc.tile_pool(name="tmpp", bufs=4))
            headp = ctx.enter_context(tc.tile_pool(name="headp", bufs=1))
            pstep = ctx.enter_context(tc.tile_pool(name="pstep", bufs=4, space="PSUM"))
            pwin = ctx.enter_context(tc.tile_pool(name="pwin", bufs=2, space="PSUM"))
            ptr = ctx.enter_context(tc.tile_pool(name="ptr", bufs=2, space="PSUM"))

            # ---- constants ----
            whb = const.tile([128, 4 * G], BF16, name="whb_s")
            nc.sync.dma_start(whb[:], whb_d[:])
            wxb = const.tile([128, 2 * G], BF16, name="wxb_s")
            nc.sync.dma_start(wxb[:], wxb_d[:])
            w1b = const.tile([128, 512], BF16, name="w1b_s")
            nc.sync.dma_start(w1b[:], w1b_d[:])
            w2b = const.tile([128, 10], BF16, name="w2b_s")
            nc.sync.dma_start(w2b[:], w2b_d[:])
            bvec = const.tile([128, 16], F32, name="bvec_s")
            nc.sync.dma_start(bvec[:], bvec_d[:])
            b1v = const.tile([128, 1], F32, name="b1v_s")
            nc.sync.dma_start(b1v[:], b1v_d[:])
            b2v = const.tile([10, 1], F32, name="b2v_s")
            nc.sync.dma_start(b2v[:], b2v_d[:])
            id128 = const.tile([128, 128], F32, name="id128_s")
            nc.sync.dma_start(id128[:], id128_d[:])
            idbf = const.tile([128, 128], BF16, name="idbf_s")
            nc.sync.dma_start(idbf[:], idbf_d[:])
            id10 = const.tile([10, 10], F32, name="id10_s")
            nc.sync.dma_start(id10[:], id10_d[:])

            # ---- state ----
            c = state.tile([128, 256], F32, name="c_s")
            nc.vector.memset(c[:], 0.0)
            h_cur = hpool.tile([128, 256], BF16, name="h_init", tag="h")
            nc.vector.memset(h_cur[:], 0.0)



            def emit_window(w):
                """Compute xzb[w]: bf16 [128, 16*512]; col m*512 + s*64 + b holds
                (Wx.T x_t + b) for gate-row m*128+p, window step s, batch b."""
                xts = [
                    xtp.tile([128, 512], BF16, name=f"xt{dk}_{w}", tag=f"xt{dk}")
                    for dk in range(2)
                ]
                for q in range(4):
                    xw = xwp.tile([128, 256], F32, name=f"xw{w}_{q}", tag="xw")
                    for two in range(2):
                        tt = w * W + 2 * q + two
                        nc.sync.dma_start(
                            xw[two * 64 : (two + 1) * 64, :], x_d[:, tt, :]
                        )
                    for dk in range(2):
                        ptile = ptr.tile(
                            [128, 128], F32, name=f"ptr{w}_{q}_{dk}", tag="ptr"
                        )
                        nc.tensor.transpose(
                            ptile[:], xw[:, dk * 128 : (dk + 1) * 128], id128[:]
                        )
                        nc.vector.tensor_copy(
                            xts[dk][:, q * 128 : (q + 1) * 128], ptile[:]
                        )
                xzb = xzp.tile([128, 16 * 512], BF16, name=f"xzb{w}", tag="xzb")
                for m in range(16):
                    pw = pwin.tile([128, 512], F32, name=f"pw{w}_{m}", tag="pw")
                    for k in range(2):
                        nc.tensor.matmul(
                            pw[:],
                            wxb[:, k * G + m * 128 : k * G + (m + 1) * 128],
                            xts[k][:],
                            start=(k == 0),
                            stop=(k == 1),
                        )
                    # bias-add + bf16 cast; alternate engines so neither
                    # ScalarE nor VectorE eats the whole drain cost
                    dst = xzb[:, m * 512 : (m + 1) * 512]
                    if m % 2 == 0:
                        nc.scalar.activation(
                            dst, pw[:], AF.Identity, bias=bvec[:, m : m + 1]
                        )
                    else:
                        nc.vector.tensor_scalar(
                            dst, pw[:], bvec[:, m : m + 1], None, ALU.add
                        )
                return xzb

            def emit_step(t, xzb, h_prev, h_new):
                s = t % W
                for grp in range(2):
                    j0 = 2 * grp
                    ps = pstep.tile([128, 512], F32, name=f"ps{t}_{grp}", tag="ps")
                    # Seed the bank with the precomputed x-projection (+bias)
                    # via one N=512 identity matmul: its start=True clears
                    # has_written bank-wide and first-writes every column, so
                    # all recurrent matmuls accumulate with start=False.  It
                    # depends only on xzb, so it runs early, during the
                    # previous step's elementwise tail.
                    xvw = xzb.rearrange("p (g j sb) -> p g j sb", g=4, j=4)
                    nc.tensor.matmul(
                        ps[:],
                        idbf[:],
                        xvw[:, :, j0 : j0 + 2, s * 64 : (s + 1) * 64],
                        start=True,
                        stop=False,
                        skip_group_check=True,
                    )
                    # k-outer order lets k=0,1 matmuls issue as soon as h
                    # chunks 0,1 are ready (before chunks 2,3 exist).
                    for k in range(4):
                        for gate in range(4):
                            for jl in range(2):
                                m = gate * 4 + j0 + jl
                                col = gate * 128 + jl * 64
                                nc.tensor.matmul(
                                    ps[:, col : col + 64],
                                    whb[:, k * G + m * 128 : k * G + (m + 1) * 128],
                                    h_prev[:, k * 64 : (k + 1) * 64],
                                    start=False,
                                    stop=(k == 3 and gate == 3 and jl == 1),
                                    skip_group_check=True,
                                )
                    zg = zp.tile([128, 512], F32, name=f"z{t}_{grp}", tag="z")
                    nc.scalar.activation(zg[:, 0:384], ps[:, 0:384], AF.Sigmoid)
                    nc.scalar.activation(zg[:, 384:512], ps[:, 384:512], AF.Tanh)
                    cg = c[:, grp * 128 : (grp + 1) * 128]
                    tmp = tmpp.tile([128, 128], F32, name=f"tmp{t}_{grp}", tag="tmp")
                    nc.vector.tensor_mul(cg, cg, zg[:, 128:256])
                    nc.vector.tensor_mul(tmp[:], zg[:, 0:128], zg[:, 384:512])
                    nc.vector.tensor_add(cg, cg, tmp[:])
                    th = tmpp.tile([128, 128], F32, name=f"th{t}_{grp}", tag="th")
                    nc.scalar.activation(th[:], cg, AF.Tanh)
                    nc.vector.tensor_mul(
                        h_new[:, grp * 128 : (grp + 1) * 128], zg[:, 256:384], th[:]
                    )

            n_windows = (t_steps + W - 1) // W
            xzbs = [None] * n_windows
            xzbs[0] = emit_window(0)
            for t in range(t_steps):
                w, s = divmod(t, W)
                if s == 0 and w + 1 < n_windows:
                    xzbs[w + 1] = emit_window(w + 1)
                h_new = hpool.tile([128, 256], BF16, name=f"h{t}", tag="h")
                emit_step(t, xzbs[w], h_cur, h_new)
                h_cur = h_new

            # ---- MLP head + softmax ----
            ps1 = ptr.tile([128, 64], F32, name="ps1", tag="ptr")
            for k in range(4):
                nc.tensor.matmul(
                    ps1[:],
                    w1b[:, k * 128 : (k + 1) * 128],
                    h_cur[:, k * 64 : (k + 1) * 64],
                    start=(k == 0),
                    stop=(k == 3),
                )
            h1 = headp.tile([128, 64], BF16, name="h1")
            nc.scalar.activation(h1[:], ps1[:], AF.Relu, bias=b1v[:, 0:1])
            ps2 = ptr.tile([10, 64], F32, name="ps2", tag="ptr")
            nc.tensor.matmul(ps2[:], w2b[:, 0:10], h1[:], start=True, stop=True)
            s2 = headp.tile([10, 64], F32, name="s2")
            nc.scalar.activation(s2[:], ps2[:], AF.Identity, bias=b2v[:, 0:1])
            ps3 = ptr.tile([64, 10], F32, name="ps3", tag="ptr")
            nc.tensor.transpose(ps3[:], s2[:], id10[:])
            z3 = headp.tile([64, 10], F32, name="z3")
            nc.vector.tensor_copy(z3[:], ps3[:])
            mx = headp.tile([64, 1], F32, name="mx")
            nc.vector.tensor_reduce(mx[:], z3[:], axis=mybir.AxisListType.X, op=ALU.max)
            nc.vector.tensor_scalar(z3[:], z3[:], mx[:, 0:1], None, ALU.subtract)
            e3 = headp.tile([64, 10], F32, name="e3")
            nc.scalar.activation(e3[:], z3[:], AF.Exp)
            sm = headp.tile([64, 1], F32, name="sm")
            nc.vector.tensor_reduce(sm[:], e3[:], axis=mybir.AxisListType.X, op=ALU.add)
            rc = headp.tile([64, 1], F32, name="rc")
            nc.vector.reciprocal(rc[:], sm[:])
            o3 = headp.tile([64, 10], F32, name="o3")
            nc.vector.tensor_scalar(o3[:], e3[:], rc[:, 0:1], None, ALU.mult)
            nc.sync.dma_start(out_d[:], o3[:])

    nc.compile()
    return nc


def _prepack(Wx, Wh, b, W1, b1, W2, b2):
    perm = np.concatenate([np.arange(512) + g * 512 for g in _GATE_PERM])
    bf = ml_dtypes.bfloat16
    Wh_p = np.ascontiguousarray(Wh[:, perm])
    Wx_p = np.ascontiguousarray(Wx[:, perm])
    b_p = np.ascontiguousarray(b[perm])
    whb = Wh_p.reshape(4, 128, G).transpose(1, 0, 2).reshape(128, 4 * G).astype(bf)
    wxb = Wx_p.reshape(2, 128, G).transpose(1, 0, 2).reshape(128, 2 * G).astype(bf)
    w1b = W1.reshape(4, 128, 128).transpose(1, 0, 2).reshape(128, 512).astype(bf)
    w2b = W2.astype(bf)
    bvec = np.ascontiguousarray(b_p.reshape(16, 128).T.astype(np.float32))
    b1v = b1.astype(np.float32).reshape(128, 1)
    b2v = b2.astype(np.float32).reshape(10, 1)
    id128 = np.eye(128, dtype=np.float32)
    idbf = np.eye(128, dtype=bf)
    id10 = np.eye(10, dtype=np.float32)
    return dict(
        whb=np.ascontiguousarray(whb),
        wxb=np.ascontiguousarray(wxb),
        w1b=np.ascontiguousarray(w1b),
        w2b=np.ascontiguousarray(w2b),
        bvec=bvec,
        b1v=b1v,
        b2v=b2v,
        id128=id128,
        idbf=idbf,
        id10=id10,
    )


_NC_CACHE = {}


def get_nc(t_steps=T):
    if t_steps not in _NC_CACHE:
        _NC_CACHE[t_steps] = _build(t_steps)
    return _NC_CACHE[t_steps]


def kernel(x, Wx, Wh, b, W1, b1, W2, b2, _trace=False):
    x = np.asarray(x, dtype=np.float32)
    consts = _prepack(
        np.asarray(Wx, np.float32),
        np.asarray(Wh, np.float32),
        np.asarray(b, np.float32),
        np.asarray(W1, np.float32),
        np.asarray(b1, np.float32),
        np.asarray(W2, np.float32),
        np.asarray(b2, np.float32),
    )
    nc = get_nc()
    in_maps = [
        {"x": np.ascontiguousarray(x[i * B : (i + 1) * B]), **consts}
        for i in range(NCORES)
    ]
    res = run_bass_kernel_spmd(nc, in_maps, core_ids=list(range(NCORES)), trace=_trace)
    out = np.concatenate([res.results[i]["out"] for i in range(NCORES)], axis=0)
    if _trace:
        return out, res
    return out


# revision 17
# speedup vs baseline: 1.4621x; 1.0261x over previous
"""Trainium2 Bass kernel for OfficeLSTM:
  h = LSTM(x)[last];  out = softmax(relu(h @ W1 + b1) @ W2 + b2)

Shapes: x [512, 256, 256] f32, Wx [256, 2048], Wh [512, 2048], b [2048],
W1 [512, 128], b1 [128], W2 [128, 10], b2 [10].  Output [512, 10] f32.

Strategy: data-parallel over 8 NeuronCores (64 batch rows each); weights
replicated (prepacked on host to bf16, gate-permuted to [i,f,o,g]).
Per-core layout keeps gates on SBUF partitions:
  z.T [2048, 64] = Wh.T @ h.T (+ xz.T), computed as 16 M-tiles x 4 K-tiles
  of [128,128]x[128,64] bf16 matmuls with Wh stationary (FWL active).
x-projections are precomputed per 8-step window with N=512 matmuls from
PE-transposed x tiles; bias is folded in via the ScalarE copy that drains
the window PSUM.  Cell state c stays f32; h is bf16.
"""

import numpy as np
import ml_dtypes
from contextlib import ExitStack

import concourse.bacc as bacc
import concourse.tile as tile
import concourse.mybir as mybir
from concourse.bass_utils import run_bass_kernel_spmd

F32 = mybir.dt.float32
BF16 = mybir.dt.bfloat16
AF = mybir.ActivationFunctionType
ALU = mybir.AluOpType

NCORES = 8
B_TOT, T, D = 512, 256, 256
U = 512
G = 4 * U            # 2048
B = B_TOT // NCORES  # 64 per core
W = 8                # window length (timesteps of xz precomputed per batch)
NW = T // W

# gate order inside the 2048 dim after host permutation: i, f, o, g
_GATE_PERM = [0, 1, 3, 2]  # new block -> original gate block


def _build(t_steps=T):
    nc = bacc.Bacc("TRN2", target_bir_lowering=False, debug=False)

    x_d = nc.declare_dram_parameter("x", [B, T, D], F32, isOutput=False)
    whb_d = nc.declare_dram_parameter("whb", [128, 4 * G], BF16, isOutput=False)
    wxb_d = nc.declare_dram_parameter("wxb", [128, 2 * G], BF16, isOutput=False)
    w1b_d = nc.declare_dram_parameter("w1b", [128, 512], BF16, isOutput=False)
    w2b_d = nc.declare_dram_parameter("w2b", [128, 10], BF16, isOutput=False)
    bvec_d = nc.declare_dram_parameter("bvec", [128, 16], F32, isOutput=False)
    b1v_d = nc.declare_dram_parameter("b1v", [128, 1], F32, isOutput=False)
    b2v_d = nc.declare_dram_parameter("b2v", [10, 1], F32, isOutput=False)
    id128_d = nc.declare_dram_parameter("id128", [128, 128], F32, isOutput=False)
    idbf_d = nc.declare_dram_parameter("idbf", [128, 128], BF16, isOutput=False)
    id10_d = nc.declare_dram_parameter("id10", [10, 10], F32, isOutput=False)
    out_d = nc.declare_dram_parameter("out", [B, 10], F32, isOutput=True)

    with tile.TileContext(nc) as tc:
        with ExitStack() as ctx:
            const = ctx.enter_context(tc.tile_pool(name="const", bufs=1))
            state = ctx.enter_context(tc.tile_pool(name="state", bufs=1))
            hpool = ctx.enter_context(tc.tile_pool(name="hpool", bufs=2))
            xwp = ctx.enter_context(tc.tile_pool(name="xwp", bufs=8))
            xtp = ctx.enter_context(tc.tile_pool(name="xtp", bufs=4))
            xzp = ctx.enter_context(tc.tile_pool(name="xzp", bufs=2))
            zp = ctx.enter_context(tc.tile_pool(name="zp", bufs=4))
            tmpp = ctx.enter_context(tc.tile_pool(name="tmpp", bufs=4))
            headp = ctx.enter_context(tc.tile_pool(name="headp", bufs=1))
            pstep = ctx.enter_context(tc.tile_pool(name="pstep", bufs=4, space="PSUM"))
            pwin = ctx.enter_context(tc.tile_pool(name="pwin", bufs=2, space="PSUM"))
            ptr = ctx.enter_context(tc.tile_pool(name="ptr", bufs=2, space="PSUM"))

            # ---- constants ----
            whb = const.tile([128, 4 * G], BF16, name="whb_s")
            nc.sync.dma_start(whb[:], whb_d[:])
            wxb = const.tile([128, 2 * G], BF16, name="wxb_s")
            nc.sync.dma_start(wxb[:], wxb_d[:])
            w1b = const.tile([128, 512], BF16, name="w1b_s")
            nc.sync.dma_start(w1b[:], w1b_d[:])
            w2b = const.tile([128, 10], BF16, name="w2b_s")
            nc.sync.dma_start(w2b[:], w2b_d[:])
            bvec = const.tile([128, 16], F32, name="bvec_s")
            nc.sync.dma_start(bvec[:], bvec_d[:])
            b1v = const.tile([128, 1], F32, name="b1v_s")
            nc.sync.dma_start(b1v[:], b1v_d[:])
            b2v = const.tile([10, 1], F32, name="b2v_s")
            nc.sync.dma_start(b2v[:], b2v_d[:])
            id128 = const.tile([128, 128], F32, name="id128_s")
            nc.sync.dma_start(id128[:], id128_d[:])
            idbf = const.tile([128, 128], BF16, name="idbf_s")
            nc.sync.dma_start(idbf[:], idbf_d[:])
            id10 = const.tile([10, 10], F32, name="id10_s")
            nc.sync.dma_start(id10[:], id10_d[:])

            # ---- state ----
            c = state.tile([128, 256], F32, name="c_s")
            nc.vector.memset(c[:], 0.0)
            h_cur = hpool.tile([128, 256], BF16, name="h_init", tag="h")
            nc.vector.memset(h_cur[:], 0.0)



            def emit_window_head(w):
                """DMA + transpose x for window w; allocate xzb."""
                xts = [
                    xtp.tile([128, 512], BF16, name=f"xt{dk}_{w}", tag=f"xt{dk}")
                    for dk in range(2)
                ]
                for q in range(4):
                    xw = xwp.tile([128, 256], F32, name=f"xw{w}_{q}", tag="xw")
                    for two in range(2):
                        tt = w * W + 2 * q + two
                        nc.sync.dma_start(
                            xw[two * 64 : (two + 1) * 64, :], x_d[:, tt, :]
                        )
                    for dk in range(2):
                        ptile = ptr.tile(
                            [128, 128], F32, name=f"ptr{w}_{q}_{dk}", tag="ptr"
                        )
                        nc.tensor.transpose(
                            ptile[:], xw[:, dk * 128 : (dk + 1) * 128], id128[:]
                        )
                        nc.vector.tensor_copy(
                            xts[dk][:, q * 128 : (q + 1) * 128], ptile[:]
                        )
                xzb = xzp.tile([128, 16 * 512], BF16, name=f"xzb{w}", tag="xzb")
                return xzb, xts

            def emit_window_chunk(w, xzb, xts, ms):
                """xz matmuls + bias/cast drain for m-tiles `ms` of window w."""
                for m in ms:
                    pw = pwin.tile([128, 512], F32, name=f"pw{w}_{m}", tag="pw")
                    for k in range(2):
                        nc.tensor.matmul(
                            pw[:],
                            wxb[:, k * G + m * 128 : k * G + (m + 1) * 128],
                            xts[k][:],
                            start=(k == 0),
                            stop=(k == 1),
                        )
                    # bias-add + bf16 cast; alternate engines so neither
                    # ScalarE nor VectorE eats the whole drain cost
                    dst = xzb[:, m * 512 : (m + 1) * 512]
                    if m % 2 == 0:
                        nc.scalar.activation(
                            dst, pw[:], AF.Identity, bias=bvec[:, m : m + 1]
                        )
                    else:
                        nc.vector.tensor_scalar(
                            dst, pw[:], bvec[:, m : m + 1], None, ALU.add
                        )

            def emit_step(t, xzb, h_prev, h_new):
                s = t % W
                for grp in range(2):
                    j0 = 2 * grp
                    ps = pstep.tile([128, 512], F32, name=f"ps{t}_{grp}", tag="ps")
                    # Seed the bank with the precomputed x-projection (+bias)
                    # via one N=512 identity matmul: its start=True clears
                    # has_written bank-wide and first-writes every column, so
                    # all recurrent matmuls accumulate with start=False.  It
                    # depends only on xzb, so it runs early, during the
                    # previous step's elementwise tail.
                    xvw = xzb.rearrange("p (g j sb) -> p g j sb", g=4, j=4)
                    nc.tensor.matmul(
                        ps[:],
                        idbf[:],
                        xvw[:, :, j0 : j0 + 2, s * 64 : (s + 1) * 64],
                        start=True,
                        stop=False,
                        skip_group_check=True,
                    )
                    # k-outer order lets k=0,1 matmuls issue as soon as h
                    # chunks 0,1 are ready (before chunks 2,3 exist).
                    for k in range(4):
                        for gate in range(4):
                            for jl in range(2):
                                m = gate * 4 + j0 + jl
                                col = gate * 128 + jl * 64
                                nc.tensor.matmul(
                                    ps[:, col : col + 64],
                                    whb[:, k * G + m * 128 : k * G + (m + 1) * 128],
                                    h_prev[:, k * 64 : (k + 1) * 64],
                                    start=False,
                                    stop=(k == 3 and gate == 3 and jl == 1),
                                    skip_group_check=True,
                                )
                    zg = zp.tile([128, 512], BF16, name=f"z{t}_{grp}", tag="z")
                    nc.scalar.activation(zg[:, 0:384], ps[:, 0:384], AF.Sigmoid)
                    nc.scalar.activation(zg[:, 384:512], ps[:, 384:512], AF.Tanh)
                    cg = c[:, grp * 128 : (grp + 1) * 128]
                    tmp = tmpp.tile([128, 128], F32, name=f"tmp{t}_{grp}", tag="tmp")
                    nc.vector.tensor_mul(cg, cg, zg[:, 128:256])
                    nc.vector.tensor_mul(tmp[:], zg[:, 0:128], zg[:, 384:512])
                    nc.vector.tensor_add(cg, cg, tmp[:])
                    th = tmpp.tile([128, 128], F32, name=f"th{t}_{grp}", tag="th")
                    nc.scalar.activation(th[:], cg, AF.Tanh)
                    nc.vector.tensor_mul(
                        h_new[:, grp * 128 : (grp + 1) * 128], zg[:, 256:384], th[:]
                    )

            n_windows = (t_steps + W - 1) // W
            xzbs = [None] * n_windows
            pending = [None] * n_windows
            xzb0, xts0 = emit_window_head(0)
            emit_window_chunk(0, xzb0, xts0, range(16))
            xzbs[0] = xzb0
            for t in range(t_steps):
                w, s = divmod(t, W)
                nw = w + 1
                if nw < n_windows:
                    if s == 0:
                        pending[nw] = emit_window_head(nw)
                        xzbs[nw] = pending[nw][0]
                    elif s in (2, 4, 6, 7):
                        m0 = {2: 0, 4: 4, 6: 8, 7: 12}[s]
                        xzb_n, xts_n = pending[nw]
                        emit_window_chunk(nw, xzb_n, xts_n, range(m0, m0 + 4))
                h_new = hpool.tile([128, 256], BF16, name=f"h{t}", tag="h")
                emit_step(t, xzbs[w], h_cur, h_new)
                h_cur = h_new

            # ---- MLP head + softmax ----
            ps1 = ptr.tile([128, 64], F32, name="ps1", tag="ptr")
            for k in range(4):
                nc.tensor.matmul(
                    ps1[:],
                    w1b[:, k * 128 : (k + 1) * 128],
                    h_cur[:, k * 64 : (k + 1) * 64],
                    start=(k == 0),
                    stop=(k == 3),
                )
            h1 = headp.tile([128, 64], BF16, name="h1")
            nc.scalar.activation(h1[:], ps1[:], AF.Relu, bias=b1v[:, 0:1])
            ps2 = ptr.tile([10, 64], F32, name="ps2", tag="ptr")
            nc.tensor.matmul(ps2[:], w2b[:, 0:10], h1[:], start=True, stop=True)
            s2 = headp.tile([10, 64], F32, name="s2")
            nc.scalar.activation(s2[:], ps2[:], AF.Identity, bias=b2v[:, 0:1])
            ps3 = ptr.tile([64, 10], F32, name="ps3", tag="ptr")
            nc.tensor.transpose(ps3[:], s2[:], id10[:])
            z3 = headp.tile([64, 10], F32, name="z3")
            nc.vector.tensor_copy(z3[:], ps3[:])
            mx = headp.tile([64, 1], F32, name="mx")
            nc.vector.tensor_reduce(mx[:], z3[:], axis=mybir.AxisListType.X, op=ALU.max)
            nc.vector.tensor_scalar(z3[:], z3[:], mx[:, 0:1], None, ALU.subtract)
            e3 = headp.tile([64, 10], F32, name="e3")
            nc.scalar.activation(e3[:], z3[:], AF.Exp)
            sm = headp.tile([64, 1], F32, name="sm")
            nc.vector.tensor_reduce(sm[:], e3[:], axis=mybir.AxisListType.X, op=ALU.add)
            rc = headp.tile([64, 1], F32, name="rc")
            nc.vector.reciprocal(rc[:], sm[:])
            o3 = headp.tile([64, 10], F32, name="o3")
            nc.vector.tensor_scalar(o3[:], e3[:], rc[:, 0:1], None, ALU.mult)
            nc.sync.dma_start(out_d[:], o3[:])

    nc.compile()
    return nc


def _prepack(Wx, Wh, b, W1, b1, W2, b2):
    perm = np.concatenate([np.arange(512) + g * 512 for g in _GATE_PERM])
    bf = ml_dtypes.bfloat16
    Wh_p = np.ascontiguousarray(Wh[:, perm])
    Wx_p = np.ascontiguousarray(Wx[:, perm])
    b_p = np.ascontiguousarray(b[perm])
    whb = Wh_p.reshape(4, 128, G).transpose(1, 0, 2).reshape(128, 4 * G).astype(bf)
    wxb = Wx_p.reshape(2, 128, G).transpose(1, 0, 2).reshape(128, 2 * G).astype(bf)
    w1b = W1.reshape(4, 128, 128).transpose(1, 0, 2).reshape(128, 512).astype(bf)
    w2b = W2.astype(bf)
    bvec = np.ascontiguousarray(b_p.reshape(16, 128).T.astype(np.float32))
    b1v = b1.astype(np.float32).reshape(128, 1)
    b2v = b2.astype(np.float32).reshape(10, 1)
    id128 = np.eye(128, dtype=np.float32)
    idbf = np.eye(128, dtype=bf)
    id10 = np.eye(10, dtype=np.float32)
    return dict(
        whb=np.ascontiguousarray(whb),
        wxb=np.ascontiguousarray(wxb),
        w1b=np.ascontiguousarray(w1b),
        w2b=np.ascontiguousarray(w2b),
        bvec=bvec,
        b1v=b1v,
        b2v=b2v,
        id128=id128,
        idbf=idbf,
        id10=id10,
    )


_NC_CACHE = {}


def get_nc(t_steps=T):
    if t_steps not in _NC_CACHE:
        _NC_CACHE[t_steps] = _build(t_steps)
    return _NC_CACHE[t_steps]


def kernel(x, Wx, Wh, b, W1, b1, W2, b2, _trace=False):
    x = np.asarray(x, dtype=np.float32)
    consts = _prepack(
        np.asarray(Wx, np.float32),
        np.asarray(Wh, np.float32),
        np.asarray(b, np.float32),
        np.asarray(W1, np.float32),
        np.asarray(b1, np.float32),
        np.asarray(W2, np.float32),
        np.asarray(b2, np.float32),
    )
    nc = get_nc()
    in_maps = [
        {"x": np.ascontiguousarray(x[i * B : (i + 1) * B]), **consts}
        for i in range(NCORES)
    ]
    res = run_bass_kernel_spmd(nc, in_maps, core_ids=list(range(NCORES)), trace=_trace)
    out = np.concatenate([res.results[i]["out"] for i in range(NCORES)], axis=0)
    if _trace:
        return out, res
    return out


# revision 19
# speedup vs baseline: 1.5303x; 1.0466x over previous
"""Trainium2 Bass kernel for OfficeLSTM:
  h = LSTM(x)[last];  out = softmax(relu(h @ W1 + b1) @ W2 + b2)

Shapes: x [512, 256, 256] f32, Wx [256, 2048], Wh [512, 2048], b [2048],
W1 [512, 128], b1 [128], W2 [128, 10], b2 [10].  Output [512, 10] f32.

Strategy: data-parallel over 8 NeuronCores (64 batch rows each); weights
replicated (prepacked on host to bf16, gate-permuted to [i,f,o,g]).
Per-core layout keeps gates on SBUF partitions:
  z.T [2048, 64] = Wh.T @ h.T (+ xz.T), computed as 16 M-tiles x 4 K-tiles
  of [128,128]x[128,64] bf16 matmuls with Wh stationary (FWL active).
x-projections are precomputed per 8-step window with N=512 matmuls from
PE-transposed x tiles; bias is folded in via the ScalarE copy that drains
the window PSUM.  Cell state c stays f32; h is bf16.
"""

import numpy as np
import ml_dtypes
from contextlib import ExitStack

import concourse.bacc as bacc
import concourse.tile as tile
import concourse.mybir as mybir
from concourse.bass_utils import run_bass_kernel_spmd

F32 = mybir.dt.float32
BF16 = mybir.dt.bfloat16
AF = mybir.ActivationFunctionType
ALU = mybir.AluOpType

NCORES = 8
B_TOT, T, D = 512, 256, 256
U = 512
G = 4 * U            # 2048
B = B_TOT // NCORES  # 64 per core
W = 8                # window length (timesteps of xz precomputed per batch)
NW = T // W

# gate order inside the 2048 dim after host permutation: i, f, o, g
_GATE_PERM = [0, 1, 3, 2]  # new block -> original gate block


def _build(t_steps=T):
    nc = bacc.Bacc("TRN2", target_bir_lowering=False, debug=False)

    x_d = nc.declare_dram_parameter("x", [B, T, D], F32, isOutput=False)
    whb_d = nc.declare_dram_parameter("whb", [128, 4 * G], BF16, isOutput=False)
    wxb_d = nc.declare_dram_parameter("wxb", [128, 2 * G], BF16, isOutput=False)
    w1b_d = nc.declare_dram_parameter("w1b", [128, 512], BF16, isOutput=False)
    w2b_d = nc.declare_dram_parameter("w2b", [128, 10], BF16, isOutput=False)
    bvec_d = nc.declare_dram_parameter("bvec", [128, 16], F32, isOutput=False)
    b1v_d = nc.declare_dram_parameter("b1v", [128, 1], F32, isOutput=False)
    b2v_d = nc.declare_dram_parameter("b2v", [10, 1], F32, isOutput=False)
    id128_d = nc.declare_dram_parameter("id128", [128, 128], F32, isOutput=False)
    idbf_d = nc.declare_dram_parameter("idbf", [128, 128], BF16, isOutput=False)
    id10_d = nc.declare_dram_parameter("id10", [10, 10], F32, isOutput=False)
    out_d = nc.declare_dram_parameter("out", [B, 10], F32, isOutput=True)

    with tile.TileContext(nc) as tc:
        with ExitStack() as ctx:
            const = ctx.enter_context(tc.tile_pool(name="const", bufs=1))
            state = ctx.enter_context(tc.tile_pool(name="state", bufs=1))
            hpool = ctx.enter_context(tc.tile_pool(name="hpool", bufs=2))
            xwp = ctx.enter_context(tc.tile_pool(name="xwp", bufs=8))
            xtp = ctx.enter_context(tc.tile_pool(name="xtp", bufs=4))
            xzp = ctx.enter_context(tc.tile_pool(name="xzp", bufs=2))
            zp = ctx.enter_context(tc.tile_pool(name="zp", bufs=4))
            tmpp = ctx.enter_context(tc.tile_pool(name="tmpp", bufs=4))
            headp = ctx.enter_context(tc.tile_pool(name="headp", bufs=1))
            pstep = ctx.enter_context(tc.tile_pool(name="pstep", bufs=6, space="PSUM"))
            pwin = ctx.enter_context(tc.tile_pool(name="pwin", bufs=2, space="PSUM"))

            # ---- constants ----
            whb = const.tile([128, 4 * G], BF16, name="whb_s")
            nc.sync.dma_start(whb[:], whb_d[:])
            wxb = const.tile([128, 2 * G], BF16, name="wxb_s")
            nc.sync.dma_start(wxb[:], wxb_d[:])
            w1b = const.tile([128, 512], BF16, name="w1b_s")
            nc.sync.dma_start(w1b[:], w1b_d[:])
            w2b = const.tile([128, 10], BF16, name="w2b_s")
            nc.sync.dma_start(w2b[:], w2b_d[:])
            bvec = const.tile([128, 16], F32, name="bvec_s")
            nc.sync.dma_start(bvec[:], bvec_d[:])
            b1v = const.tile([128, 1], F32, name="b1v_s")
            nc.sync.dma_start(b1v[:], b1v_d[:])
            b2v = const.tile([10, 1], F32, name="b2v_s")
            nc.sync.dma_start(b2v[:], b2v_d[:])
            id128 = const.tile([128, 128], F32, name="id128_s")
            nc.sync.dma_start(id128[:], id128_d[:])
            idbf = const.tile([128, 128], BF16, name="idbf_s")
            nc.sync.dma_start(idbf[:], idbf_d[:])
            id10 = const.tile([10, 10], F32, name="id10_s")
            nc.sync.dma_start(id10[:], id10_d[:])

            # ---- state ----
            c = state.tile([128, 256], F32, name="c_s")
            nc.vector.memset(c[:], 0.0)
            h_cur = hpool.tile([128, 256], BF16, name="h_init", tag="h")
            nc.vector.memset(h_cur[:], 0.0)



            def emit_window_head(w):
                """DMA + transpose x for window w; allocate xzb."""
                xts = [
                    xtp.tile([128, 512], BF16, name=f"xt{dk}_{w}", tag=f"xt{dk}")
                    for dk in range(2)
                ]
                for q in range(4):
                    xw = xwp.tile([128, 256], F32, name=f"xw{w}_{q}", tag="xw")
                    for two in range(2):
                        tt = w * W + 2 * q + two
                        nc.sync.dma_start(
                            xw[two * 64 : (two + 1) * 64, :], x_d[:, tt, :]
                        )
                    for dk in range(2):
                        ptile = pwin.tile(
                            [128, 128], F32, name=f"ptr{w}_{q}_{dk}", tag="pw"
                        )
                        nc.tensor.transpose(
                            ptile[:], xw[:, dk * 128 : (dk + 1) * 128], id128[:]
                        )
                        nc.vector.tensor_copy(
                            xts[dk][:, q * 128 : (q + 1) * 128], ptile[:]
                        )
                xzb = xzp.tile([128, 16 * 512], BF16, name=f"xzb{w}", tag="xzb")
                return xzb, xts

            def emit_window_chunk(w, xzb, xts, ms):
                """xz matmuls + bias/cast drain for m-tiles `ms` of window w."""
                for m in ms:
                    pw = pwin.tile([128, 512], F32, name=f"pw{w}_{m}", tag="pw")
                    for k in range(2):
                        nc.tensor.matmul(
                            pw[:],
                            wxb[:, k * G + m * 128 : k * G + (m + 1) * 128],
                            xts[k][:],
                            start=(k == 0),
                            stop=(k == 1),
                        )
                    # bias-add + bf16 cast; alternate engines so neither
                    # ScalarE nor VectorE eats the whole drain cost
                    dst = xzb[:, m * 512 : (m + 1) * 512]
                    if m % 2 == 0:
                        nc.scalar.activation(
                            dst, pw[:], AF.Identity, bias=bvec[:, m : m + 1]
                        )
                    else:
                        nc.vector.tensor_scalar(
                            dst, pw[:], bvec[:, m : m + 1], None, ALU.add
                        )

            ps_tiles = {}

            def emit_seed(t, xzb):
                """Allocate step-t PSUM banks and seed them with the
                precomputed x-projection (+bias) via one N=512 identity
                matmul per group: its start=True clears has_written
                bank-wide and first-writes every column, so all recurrent
                matmuls accumulate with start=False.  Seeds depend only on
                xzb, so emitting them ahead of the previous step's matmuls
                lets the PE fill its h-dependency stall with them."""
                s = t % W
                xvw = xzb.rearrange("p (g j sb) -> p g j sb", g=4, j=4)
                pr = []
                for grp in range(2):
                    j0 = 2 * grp
                    ps = pstep.tile([128, 512], F32, name=f"ps{t}_{grp}", tag="ps")
                    nc.tensor.matmul(
                        ps[:],
                        idbf[:],
                        xvw[:, :, j0 : j0 + 2, s * 64 : (s + 1) * 64],
                        start=True,
                        stop=False,
                        skip_group_check=True,
                    )
                    pr.append(ps)
                ps_tiles[t] = pr

            def emit_step(t, xzb, h_prev, h_new):
                s = t % W
                for grp in range(2):
                    j0 = 2 * grp
                    ps = ps_tiles.pop(t)[grp] if grp == 1 else ps_tiles[t][grp]
                    # k-outer order lets k=0,1 matmuls issue as soon as h
                    # chunks 0,1 are ready (before chunks 2,3 exist).
                    for k in range(4):
                        for gate in range(4):
                            for jl in range(2):
                                m = gate * 4 + j0 + jl
                                col = gate * 128 + jl * 64
                                nc.tensor.matmul(
                                    ps[:, col : col + 64],
                                    whb[:, k * G + m * 128 : k * G + (m + 1) * 128],
                                    h_prev[:, k * 64 : (k + 1) * 64],
                                    start=False,
                                    stop=(k == 3 and gate == 3 and jl == 1),
                                    skip_group_check=True,
                                )
                    zg = zp.tile([128, 512], BF16, name=f"z{t}_{grp}", tag="z")
                    nc.scalar.activation(zg[:, 0:384], ps[:, 0:384], AF.Sigmoid)
                    nc.scalar.activation(zg[:, 384:512], ps[:, 384:512], AF.Tanh)
                    cg = c[:, grp * 128 : (grp + 1) * 128]
                    tmp = tmpp.tile([128, 128], BF16, name=f"tmp{t}_{grp}", tag="tmp")
                    nc.vector.tensor_mul(cg, cg, zg[:, 128:256])
                    nc.vector.tensor_mul(tmp[:], zg[:, 0:128], zg[:, 384:512])
                    nc.vector.tensor_add(cg, cg, tmp[:])
                    th = tmpp.tile([128, 128], F32, name=f"th{t}_{grp}", tag="th")
                    nc.scalar.activation(th[:], cg, AF.Tanh)
                    nc.vector.tensor_mul(
                        h_new[:, grp * 128 : (grp + 1) * 128], zg[:, 256:384], th[:]
                    )

            n_windows = (t_steps + W - 1) // W
            xzbs = [None] * n_windows
            pending = [None] * n_windows
            xzb0, xts0 = emit_window_head(0)
            emit_window_chunk(0, xzb0, xts0, range(16))
            xzbs[0] = xzb0
            emit_seed(0, xzb0)
            for t in range(t_steps):
                w, s = divmod(t, W)
                nw = w + 1
                if nw < n_windows:
                    if s == 0:
                        pending[nw] = emit_window_head(nw)
                        xzbs[nw] = pending[nw][0]
                    elif s in (2, 4, 6, 7):
                        m0 = {2: 0, 4: 4, 6: 8, 7: 12}[s]
                        xzb_n, xts_n = pending[nw]
                        emit_window_chunk(nw, xzb_n, xts_n, range(m0, m0 + 4))
                if t + 1 < t_steps:
                    emit_seed(t + 1, xzbs[(t + 1) // W])
                h_new = hpool.tile([128, 256], BF16, name=f"h{t}", tag="h")
                emit_step(t, xzbs[w], h_cur, h_new)
                h_cur = h_new

            # ---- MLP head + softmax ----
            ps1 = pwin.tile([128, 64], F32, name="ps1", tag="pw")
            for k in range(4):
                nc.tensor.matmul(
                    ps1[:],
                    w1b[:, k * 128 : (k + 1) * 128],
                    h_cur[:, k * 64 : (k + 1) * 64],
                    start=(k == 0),
                    stop=(k == 3),
                )
            h1 = headp.tile([128, 64], BF16, name="h1")
            nc.scalar.activation(h1[:], ps1[:], AF.Relu, bias=b1v[:, 0:1])
            ps2 = pwin.tile([10, 64], F32, name="ps2", tag="pw")
            nc.tensor.matmul(ps2[:], w2b[:, 0:10], h1[:], start=True, stop=True)
            s2 = headp.tile([10, 64], F32, name="s2")
            nc.scalar.activation(s2[:], ps2[:], AF.Identity, bias=b2v[:, 0:1])
            ps3 = pwin.tile([64, 10], F32, name="ps3", tag="pw")
            nc.tensor.transpose(ps3[:], s2[:], id10[:])
            z3 = headp.tile([64, 10], F32, name="z3")
            nc.vector.tensor_copy(z3[:], ps3[:])
            mx = headp.tile([64, 1], F32, name="mx")
            nc.vector.tensor_reduce(mx[:], z3[:], axis=mybir.AxisListType.X, op=ALU.max)
            nc.vector.tensor_scalar(z3[:], z3[:], mx[:, 0:1], None, ALU.subtract)
            e3 = headp.tile([64, 10], F32, name="e3")
            nc.scalar.activation(e3[:], z3[:], AF.Exp)
            sm = headp.tile([64, 1], F32, name="sm")
            nc.vector.tensor_reduce(sm[:], e3[:], axis=mybir.AxisListType.X, op=ALU.add)
            rc = headp.tile([64, 1], F32, name="rc")
            nc.vector.reciprocal(rc[:], sm[:])
            o3 = headp.tile([64, 10], F32, name="o3")
            nc.vector.tensor_scalar(o3[:], e3[:], rc[:, 0:1], None, ALU.mult)
            nc.sync.dma_start(out_d[:], o3[:])

    nc.compile()
    return nc


def _prepack(Wx, Wh, b, W1, b1, W2, b2):
    perm = np.concatenate([np.arange(512) + g * 512 for g in _GATE_PERM])
    bf = ml_dtypes.bfloat16
    Wh_p = np.ascontiguousarray(Wh[:, perm])
    Wx_p = np.ascontiguousarray(Wx[:, perm])
    b_p = np.ascontiguousarray(b[perm])
    whb = Wh_p.reshape(4, 128, G).transpose(1, 0, 2).reshape(128, 4 * G).astype(bf)
    wxb = Wx_p.reshape(2, 128, G).transpose(1, 0, 2).reshape(128, 2 * G).astype(bf)
    w1b = W1.reshape(4, 128, 128).transpose(1, 0, 2).reshape(128, 512).astype(bf)
    w2b = W2.astype(bf)
    bvec = np.ascontiguousarray(b_p.reshape(16, 128).T.astype(np.float32))
    b1v = b1.astype(np.float32).reshape(128, 1)
    b2v = b2.astype(np.float32).reshape(10, 1)
    id128 = np.eye(128, dtype=np.float32)
    idbf = np.eye(128, dtype=bf)
    id10 = np.eye(10, dtype=np.float32)
    return dict(
        whb=np.ascontiguousarray(whb),
        wxb=np.ascontiguousarray(wxb),
        w1b=np.ascontiguousarray(w1b),
        w2b=np.ascontiguousarray(w2b),
        bvec=bvec,
        b1v=b1v,
        b2v=b2v,
        id128=id128,
        idbf=idbf,
        id10=id10,
    )


_NC_CACHE = {}


def get_nc(t_steps=T):
    if t_steps not in _NC_CACHE:
        _NC_CACHE[t_steps] = _build(t_steps)
    return _NC_CACHE[t_steps]


def kernel(x, Wx, Wh, b, W1, b1, W2, b2, _trace=False):
    x = np.asarray(x, dtype=np.float32)
    consts = _prepack(
        np.asarray(Wx, np.float32),
        np.asarray(Wh, np.float32),
        np.asarray(b, np.float32),
        np.asarray(W1, np.float32),
        np.asarray(b1, np.float32),
        np.asarray(W2, np.float32),
        np.asarray(b2, np.float32),
    )
    nc = get_nc()
    in_maps = [
        {"x": np.ascontiguousarray(x[i * B : (i + 1) * B]), **consts}
        for i in range(NCORES)
    ]
    res = run_bass_kernel_spmd(nc, in_maps, core_ids=list(range(NCORES)), trace=_trace)
    out = np.concatenate([res.results[i]["out"] for i in range(NCORES)], axis=0)
    if _trace:
        return out, res
    return out
